# revision 1
# baseline (speedup 1.0000x reference)
"""Trainium2 Bass kernel for DGLFeatureGAT (dense GATv2 over complete graph).

Reference computation (per batch b, head h; N=64 nodes, D=128 feat dim):
    el = xn @ Wl,  er = xn @ Wr                      # [N, H, D]
    e[h,i,j] = sum_d a[h,d] * lrelu(el[j,h,d] + er[i,h,d])
    alpha = softmax_j(e);  rst[i,h,d] = sum_j alpha[h,i,j] el[j,h,d] + bias
    out = mean_h(rst) transposed to [D, N]

Exact decomposition (slope s=0.2, c1=(1+s)/2, c2=(1-s)/2):
    e = c2*sum_d a_d|z_d| + c1*u_j + c1*v_i          # z = el_j + er_i
      - v_i constant over j -> dropped (softmax invariant)
      - u_j folded into the projection matmul (2 extra columns); enters the
        softmax as g_j = exp(c1*u_j) multiplied into the aggregation rhs,
        with one extra rhs column accumulating the softmax normalizer.

On-chip pipeline per (b, h) "unit":
    PE:   z[d, (j,i)] = [el;er]-stacked stationary x constant 0/1 selector
          (f32r, 1 cyc/col, j-major columns); e-reduce with 32x-replicated
          c2*a_h stationary, chunk c -> PSUM tile c%4 at base 32*(c//4);
          fp32 aggregation pT.T @ [el*g | g].
    DVE:  |z| via tensor_scalar(abs_max, 0); el*g; reciprocal; normalize.
    ACT:  exp(u); exp(e) straight out of PSUM (this is also the psum->sbuf
          stage); final bias add.  j-major => exp output IS p-transposed.
    DMA:  one input blob (HWDGE), per-unit gather/reshape (SWDGE), y out.

Sharding: pure data-parallel, B=32 -> 4 batches per core x 8 cores.
"""

import numpy as np
from contextlib import ExitStack

import concourse.bass as bass
import concourse.bacc as bacc
import concourse.tile as tile
from concourse import mybir
from concourse.bass_utils import run_bass_kernel_spmd

f32 = mybir.dt.float32
f32r = mybir.dt.float32r
Act = mybir.ActivationFunctionType

B, W, F, H, D = 32, 128, 64, 2, 128
NEG_SLOPE = 0.2
C1 = (1.0 + NEG_SLOPE) / 2.0   # 0.6
C2 = (1.0 - NEG_SLOPE) / 2.0   # 0.4
N_CORES = 8
B_LOC = B // N_CORES            # 4 batches per core
N = F                           # 64 nodes
NCHUNK = 8                      # 512-col chunks of the (j,i)=4096 space

# blob column layout (float32 bits, declared f32r)
OFF_X = 0                           # [128, B_LOC*64]: x[b] stacked (lhsT)
OFF_WLR = OFF_X + B_LOC * N         # [128, 514]: Wl | Wr | wl_u
OFF_SSEL = OFF_WLR + 2 * H * D + 2  # [128, 2048]: selector bf16-packed
OFF_AREP = OFF_SSEL + N * N // 2    # [128, 64]: a_h replicated 32x per h
OFF_AREPB = OFF_AREP + 2 * 32       # [128, 32]: a_rep packed as bf16 pairs
OFF_BIAS = OFF_AREPB + 32           # [128, 1]: fused output bias
OFF_I64 = OFF_BIAS + 1              # [128, 64]: identity (rows 0..63)
NCOLS = OFF_I64 + N

_cache = {}


def _build():
    if "nc" in _cache:
        return _cache["nc"]
    nc = bacc.Bacc("TRN2", target_bir_lowering=False, debug=False)
    blob_d = nc.declare_dram_parameter("blob", [128, NCOLS], f32,
                                       isOutput=False).ap()
    y_d = nc.declare_dram_parameter("y", [B_LOC, D, F], f32,
                                    isOutput=True).ap()

    with tile.TileContext(nc) as tc, ExitStack() as ctx:
        sb1 = ctx.enter_context(tc.tile_pool(name="sb1", bufs=1))
        sbE = ctx.enter_context(tc.tile_pool(name="sbE", bufs=B_LOC))
        sbZ = ctx.enter_context(tc.tile_pool(name="sbZ", bufs=2 * B_LOC))
        sbS = ctx.enter_context(tc.tile_pool(name="sbS", bufs=2))
        sbU = ctx.enter_context(tc.tile_pool(name="sbU", bufs=B_LOC))
        psZ = ctx.enter_context(tc.tile_pool(name="psZ", bufs=2, space="PSUM"))
        psR = ctx.enter_context(tc.tile_pool(name="psR", bufs=4, space="PSUM"))
        psS = ctx.enter_context(tc.tile_pool(name="psS", bufs=2, space="PSUM"))

        blob = sb1.tile([128, NCOLS], f32, tag="blob")
        nc.sync.dma_start(blob[:], blob_d)

        def bl(off, w):
            return blob[:, off:off + w]

        ident = blob[0:N, OFF_I64:OFF_I64 + N]
        bias_ap = bl(OFF_BIAS, 1)

        bf16 = mybir.dt.bfloat16

        y_all = sb1.tile([D, B_LOC * N], f32, tag="yall")
        pair_elgs = {}
        for b in range(B_LOC):
            xb = blob[:, OFF_X + b * N:OFF_X + (b + 1) * N]  # [128,64] lhsT

            proj = psS.tile([N, 512], f32, tag="sm", name="proj")
            nc.tensor.matmul(proj[:], xb, bl(OFF_WLR, 512),
                             start=True, stop=True)
            proju = psS.tile([N, 2], f32, tag="sm", name="proju")
            nc.tensor.matmul(proju[:], xb, bl(OFF_WLR + 512, 2),
                             start=True, stop=True)

            # stacked [el; er] (rows 0:64 el, 64:128 er), f32r for z-form
            eler = sbE.tile([128, H * D], bf16, tag="eler")
            nc.vector.tensor_copy(eler[0:N, :], proj[:, 0:H * D])
            nc.vector.tensor_copy(eler[N:128, :], proj[:, H * D:2 * H * D])

            g_b = sbU.tile([N, H], f32, tag="g")   # g[j,h] = exp(c1*u)
            nc.scalar.activation(g_b[:], proju[:], Act.Exp)

            # elg[j, 0:D] = el[j, :] * g_j ; elg[j, D] = g_j   (DVE)
            elgs = []
            for h in range(H):
                elg = sbU.tile([N, D + 1], f32, tag=f"elg{b}{h}",
                               name=f"elg{b}{h}", bufs=1)
                nc.vector.tensor_scalar(
                    elg[:, 0:D], proj[:, h * D:(h + 1) * D],
                    g_b[:, h:h + 1], None, mybir.AluOpType.mult)
                nc.vector.tensor_copy(elg[:, D:D + 1], g_b[:, h:h + 1])
                elgs.append(elg)
            pair_elgs[b] = elgs

            if b == 0:
                staged = sbS.tile([N, 4 * 4096], f32, tag="staged",
                                  name="staged", bufs=1)
            for h in range(H):
                zabs = sbZ.tile([128, N * N], bf16, tag="zabs")
                # chunk c -> R tile c%4, partition base 32*(c//4)
                R = [psR.tile([N, 512], f32, tag="ru", name=f"R{k}")
                     for k in range(4)]
                for c in range(NCHUNK):
                    zc = psZ.tile([128, 512], f32, tag="zc")
                    nc.tensor.matmul(
                        zc[:], eler[:, h * D:(h + 1) * D],
                        bl(OFF_SSEL, N * N // 2).bitcast(bf16)
                        [:, 512 * c:512 * (c + 1)],
                        start=True, stop=True)
                    nc.vector.tensor_scalar(
                        zabs[:, 512 * c:512 * (c + 1)], zc[:], 0.0, None,
                        mybir.AluOpType.max)
                    base = 32 * (c // 4)
                    nc.tensor.matmul(
                        R[c % 4][base:base + 32, :],
                        bl(OFF_AREPB, 32).bitcast(bf16)[:, 32 * h:32 * (h + 1)],
                        zabs[:, 512 * c:512 * (c + 1)],
                        start=True, stop=True,
                        skip_group_check=True)

                # exp straight out of PSUM; j-major => this is p TRANSPOSED
                # staged layout: f = 4096*t + 512*j_lo + 128*b + 64*h + i
                sview = staged[:].rearrange("p (t j e) -> p t j e", t=4, e=512)
                off = 128 * b + 64 * h
                for t in range(4):
                    nc.scalar.activation(
                        sview[:, t, :, off:off + 64],
                        R[t][:].rearrange("p (j i) -> p j i", i=64),
                        Act.Exp)


        # ---- tail: gather pT2, aggregate, normalize, output ----
        # pT2 [j, (b, h, i)]: j = 32*s + 8*t + j_lo from
        # staged[32*s + rep, 4096*t + 512*j_lo + (128*b + 64*h + i)]
        pT2 = sbU.tile([N, B_LOC * H * N], f32, tag="pT2", bufs=1)
        src_ap = bass.AP(
            tensor=staged.tensor, offset=staged.offset,
            ap=[[32 * 16384, 2], [4096, 4], [512, 8], [1, 512]])
        nc.sync.dma_start(pT2[:], src_ap)

        for b in range(B_LOC):
            t_parts = []
            for h in range(H):
                ag = psS.tile([N, D + 1], f32, tag="sm", name="ag")
                nc.tensor.matmul(
                    ag[:], pT2[:, 128 * b + 64 * h:128 * b + 64 * (h + 1)],
                    pair_elgs[b][h][:], start=True, stop=True)

                r_u = sbU.tile([N, 1], f32, tag="r")
                nc.vector.reciprocal(r_u[:], ag[:, D:D + 1])
                rh = sbU.tile([N, 1], f32, tag="rh")
                nc.vector.tensor_scalar(rh[:], r_u[:], 0.5, None,
                                        mybir.AluOpType.mult)
                t_h = sbU.tile([N, D], f32, tag="th", name=f"th{h}")
                nc.vector.tensor_scalar(t_h[:], ag[:, 0:D], rh[:], None,
                                        mybir.AluOpType.mult)
                t_parts.append(t_h)

            tsum = sbU.tile([N, D], f32, tag="tsum")
            nc.vector.tensor_tensor(tsum[:], t_parts[0][:], t_parts[1][:],
                                    mybir.AluOpType.add)
            oT = psS.tile([D, N], f32, tag="sm", name="oT")
            nc.tensor.transpose(oT[:], tsum[:], ident)
            nc.scalar.activation(y_all[:, N * b:N * (b + 1)], oT[:],
                                 Act.Identity, bias=bias_ap)

        # single output DMA: y_all[d, (b, f)] -> y[b, d, f]
        y_src = bass.AP(tensor=y_all.tensor, offset=y_all.offset,
                        ap=[[B_LOC * N, 128], [N, B_LOC], [1, N]])
        y_dst = bass.AP(tensor=y_d.tensor, offset=y_d.offset,
                        ap=[[N, 128], [128 * N, B_LOC], [1, N]])
        nc.sync.dma_start(y_dst, y_src)

    nc.compile()
    _cache["nc"] = nc
    return nc


def _make_blobs(x, Wl, Wr, attn_a, bias):
    """Host-side prep: per-core input blobs [128, NCOLS] float32."""
    x = np.asarray(x, np.float32)
    Wl = np.asarray(Wl, np.float32)
    Wr = np.asarray(Wr, np.float32)
    attn_a = np.asarray(attn_a, np.float32)
    bias = np.asarray(bias, np.float32)

    import ml_dtypes
    wl_u = np.einsum("whd,hd->wh", Wl.reshape(W, H, D), attn_a) * NEG_SLOPE
    wlr = np.concatenate([Wl, Wr, wl_u], axis=1)                  # [128, 514]

    # selector: chunk c (512 cols) covers j-block (c+4)%8 (so the base-32
    # PSUM write comes first within each bank's accumulation group)
    s_sel = np.zeros((128, N * N), np.float32)
    for c in range(NCHUNK):
        jb = c
        loc = np.arange(512)
        cols = 512 * c + loc
        j_idx = 8 * jb + loc // N
        i_idx = loc % N
        s_sel[j_idx, cols] = 1.0      # rows 0..63 (el side): select j
        s_sel[N + i_idx, cols] = 1.0  # rows 64..127 (er side): select i

    a_rep = np.concatenate(
        [np.repeat(((1.0 - NEG_SLOPE) * attn_a[h]).reshape(128, 1), 32, axis=1)
         for h in range(H)], axis=1)                              # [128, 64]
    a_bf = a_rep.astype(ml_dtypes.bfloat16).view(np.uint16)      # [128, 64]
    a_packed = (a_bf[:, 0::2].astype(np.uint32)
                | (a_bf[:, 1::2].astype(np.uint32) << 16)).view(np.float32)
    bias_f = 0.5 * (bias.reshape(H, D)[0] + bias.reshape(H, D)[1])
    ident = np.zeros((128, N), np.float32)
    ident[0:N, :] = np.eye(N, dtype=np.float32)

    s_bf = s_sel.astype(ml_dtypes.bfloat16).view(np.uint16)
    s_packed = (s_bf[:, 0::2].astype(np.uint32)
                | (s_bf[:, 1::2].astype(np.uint32) << 16)).view(np.float32)
    common = np.concatenate(
        [wlr, s_packed, a_rep, a_packed, bias_f.reshape(128, 1), ident],
        axis=1)

    blobs = []
    for c in range(N_CORES):
        xs = x[c * B_LOC:(c + 1) * B_LOC]          # [4, 128, 64]
        xsec = xs.transpose(1, 0, 2).reshape(128, B_LOC * N)
        blobs.append(np.ascontiguousarray(
            np.concatenate([xsec, common], axis=1)))
    return blobs


def kernel(x, Wl, Wr, attn_a, bias):
    nc = _build()
    blobs = _make_blobs(x, Wl, Wr, attn_a, bias)
    in_maps = [{"blob": blobs[c]} for c in range(N_CORES)]
    res = run_bass_kernel_spmd(nc, in_maps, list(range(N_CORES)))
    out = np.concatenate([res.results[c]["y"] for c in range(N_CORES)],
                         axis=0)
    return out.astype(np.float32)



# revision 17
# speedup vs baseline: 1.4548x; 1.4548x over previous
"""Trainium2 Bass kernel for DGLFeatureGAT (dense GATv2, complete graph).

Math (per batch b, head h; N=64 nodes, D=128, lrelu slope s=0.2):
    el = xn @ Wl, er = xn @ Wr;  z[d,(j,i)] = el[j,d] + er[i,d]
    e[i,j] = s*u_j + (1-s)*sum_d a_d relu(z)   (+ const_i, dropped in softmax)
    alpha = softmax_j(e * ...);  out = mean_h(rst) + bias, transposed.
    u_j enters as g_j = exp(s*u_j) multiplied into the aggregation rhs.

v2 pipeline (engine-balanced, TimelineSim-cost-model driven):
    PE : bf16 projection (1cyc/col); z via [el;er]-stacked x 0/1 selector
         (bf16, or fp8e4 DoubleRow at 0.5cyc/col when USE_FP8); e-reduce with
         32x-replicated 0.8*a_h -> R[t]=[128,512] PSUM (4 chunks per tile);
         bf16 aggregation; output transpose.
    DVE/Act/Pool : relu chunks round-robin; Act also exp([128,512]) + final
         bias (scale=0.5 folds the head-mean); Pool also elg=el*g.
    DMA: split input blobs (weights first), per-b pT gather, per-b y out.

Sharding: pure data-parallel, B=32 -> 4 batches x 8 cores.
"""

import numpy as np
from contextlib import ExitStack

import concourse.bass as bass
import concourse.bacc as bacc
import concourse.tile as tile
from concourse import mybir
from concourse.bass_utils import run_bass_kernel_spmd

f32 = mybir.dt.float32
bf16 = mybir.dt.bfloat16
f8e4 = mybir.dt.float8e4
Act = mybir.ActivationFunctionType
Alu = mybir.AluOpType

B, W, F, H, D = 32, 128, 64, 2, 128
NEG_SLOPE = 0.2
N_CORES = 8
B_LOC = B // N_CORES
N = F
NCHUNK = 8

USE_FP8 = False

# blobA layout (f32 cols)
OFF_X = 0                       # [128,128]: x bf16-packed (256 bf16, lhsT)
OFF_WLR = OFF_X + 128           # [128,257]: Wl|Wr|wl_u bf16-packed (514 bf16)
OFF_AREPB = OFF_WLR + 257       # [128,32]: 0.8*a 32x-replicated, bf16 pairs
OFF_BIAS = OFF_AREPB + 32       # [128,1]: fused output bias (f32)
OFF_I64 = OFF_BIAS + 1          # [128,64]: f32 identity (rows 0..63)
NA = OFF_I64 + N

_cache = {}


def _build():
    key = ("nc", USE_FP8)
    if key in _cache:
        return _cache[key]
    nc = bacc.Bacc("TRN2", target_bir_lowering=False, debug=False)
    blobA_d = nc.declare_dram_parameter("blobA", [128, NA], f32,
                                        isOutput=False).ap()
    nb = 2048 if USE_FP8 else 2048
    selp = 64 if USE_FP8 else 128
    blobB_d = nc.declare_dram_parameter("blobB", [selp, nb], f32,
                                        isOutput=False).ap()
    y_d = nc.declare_dram_parameter("y", [B_LOC, D, F], f32,
                                    isOutput=True).ap()

    # relu chunk -> engine assignment
    RELU_ENG = ["D", "A", "D", "A", "D", "A", "D", "D"]
    # chunk c -> exp tile v, 32-row base m, j-range [JB[c], JB[c]+8)
    CVM = [(c % 3, c // 3) if c < 6 else (c - 6, 2) for c in range(NCHUNK)]
    JB = [24 * m + 8 * v if m < 2 else 48 + 8 * v for (v, m) in CVM]

    with tile.TileContext(nc) as tc, ExitStack() as ctx:
        sb1 = ctx.enter_context(tc.tile_pool(name="sb1", bufs=1))
        sbE = ctx.enter_context(tc.tile_pool(name="sbE", bufs=2))
        sbZ = ctx.enter_context(tc.tile_pool(name="sbZ", bufs=2))
        sbS = ctx.enter_context(tc.tile_pool(name="sbS", bufs=2))
        sbU = ctx.enter_context(tc.tile_pool(name="sbU", bufs=B_LOC))
        psP = ctx.enter_context(tc.tile_pool(name="psP", bufs=1, space="PSUM"))
        psZ = ctx.enter_context(tc.tile_pool(name="psZ", bufs=3, space="PSUM"))
        psR = ctx.enter_context(tc.tile_pool(name="psR", bufs=4, space="PSUM"))

        warm = sb1.tile([1, 1], f32, tag="warm")
        nc.scalar.activation(warm[:], warm[:], Act.Exp)

        blobA = sb1.tile([128, NA], f32, tag="blobA")
        nc.sync.dma_start(blobA[:], blobA_d)
        blobB = sb1.tile([selp, nb], f32, tag="blobB")
        for q in range(4):
            nc.sync.dma_start(blobB[:, 512 * q:512 * (q + 1)],
                              blobB_d[:, 512 * q:512 * (q + 1)])

        x_bf = blobA[:, OFF_X:OFF_X + 128].bitcast(bf16)         # [128,256]
        wlr = blobA[:, OFF_WLR:OFF_WLR + 257].bitcast(bf16)      # [128,514]
        a_bf = blobA[:, OFF_AREPB:OFF_AREPB + 32].bitcast(bf16)  # [128,64]
        bias_ap = blobA[:, OFF_BIAS:OFF_BIAS + 1]
        ident = blobA[0:N, OFF_I64:OFF_I64 + N]

        if USE_FP8:
            sel8 = blobB[:].bitcast(f8e4).rearrange(
                "p (t ji) -> p t ji", t=2)                       # [64,2,4096]
        else:
            selb = blobB[:].bitcast(bf16)                        # [128,4096]

        y_all = sb1.tile([D, B_LOC * N], f32, tag="yall")

        state = {}   # per-b tiles needed by the delayed tail

        deferred = []   # software pipeline: reduce/exp lag z/relu by LA
        LA = 4

        def pump(keep):
            while len(deferred) > keep:
                deferred.pop(0)()

        def emit_unit(b, h, P1, eler, g_b):
            """z chunks + relu queued now; e-reduce/exp deferred by LA.

            chunk c covers j in [JB[c], JB[c]+8) -> R[v] rows [32m, 32m+32);
            staged free layout (v, j_lo, h, i): partition-block m reads as one
            contiguous run in j-order.
            """
            NP = [96, 96, 64]
            R = [psR.tile([NP[v], 512], f32, tag="ru", name=f"R{b}{h}{v}")
                 for v in range(3)]
            zabs = sbZ.tile([128, N * N], bf16, tag="zabs")
            staged = state[("staged", b)]
            sview = staged[:].rearrange("p (v j h2 i) -> p v j h2 i",
                                        v=3, j=8, h2=2)
            if USE_FP8:
                lhs = eler[:].rearrange("p (t h2 d) -> p t h2 d",
                                        t=2, h2=2)[:, :, h, :]
            else:
                lhs = eler[:, h * D:(h + 1) * D]
            for c in range(NCHUNK):
                zc = psZ.tile([128, 512], f32, tag="zc")
                if USE_FP8:
                    nc.tensor.matmul(
                        zc[:], lhs, sel8[:, :, 512 * c:512 * (c + 1)],
                        start=True, stop=True,
                        perf_mode=mybir.MatmulPerfMode.DoubleRow)
                else:
                    nc.tensor.matmul(
                        zc[:], lhs, selb[:, 512 * c:512 * (c + 1)],
                        start=True, stop=True)
                zs = zabs[:, 512 * c:512 * (c + 1)]
                eng = RELU_ENG[c]
                if eng == "D":
                    nc.vector.tensor_scalar(zs, zc[:], 0.0, None, Alu.max)
                elif eng == "A":
                    nc.scalar.activation(zs, zc[:], Act.Relu)
                else:
                    nc.gpsimd.tensor_scalar(zs, zc[:], 0.0, None, Alu.max)

                def red(c=c, zs=zs, h=h):
                    v, m = CVM[c]
                    nc.tensor.matmul(
                        R[v][32 * m:32 * m + 32, :],
                        a_bf[:, 32 * h:32 * h + 32],
                        zs, start=True, stop=True, skip_group_check=True)
                    v2 = {6: 0, 7: 1, 5: 2}.get(c)
                    if v2 is not None:
                        nc.scalar.activation(
                            sview[0:NP[v2], v2, :, h, :],
                            R[v2][:].rearrange("p (j i) -> p j i", i=64),
                            Act.Exp)
                deferred.append(red)
                pump(LA)

        def emit_head(b):
            """projection + eler for batch b (psP: own bank, prefetchable)."""
            xb = x_bf[:, N * b:N * (b + 1)]
            P1 = psP.tile([N, 512], f32, tag="p1", name=f"P1_{b}")
            nc.tensor.matmul(P1[:], xb, wlr[:, 0:512], start=True, stop=True)
            if USE_FP8:
                eler = sbE.tile([N, 512], f8e4, tag="eler")
                nc.vector.tensor_copy(eler[:], P1[:])
            else:
                eler = sbE.tile([128, H * D], bf16, tag="eler")
                nc.vector.tensor_copy(eler[0:N, :], P1[:, 0:H * D])
                nc.scalar.copy(eler[N:128, :], P1[:, H * D:2 * H * D])
            staged = sbS.tile([128, 3072], bf16, tag="staged",
                              name=f"staged{b}")
            state[("staged", b)] = staged
            state[("P1", b)] = P1
            return P1, eler, None

        def emit_mid(b, eler):
            """proju + g + elg, emitted between the two units of b."""
            xb = x_bf[:, N * b:N * (b + 1)]
            P2 = psR.tile([N, 2], f32, tag="ru", name=f"P2_{b}")
            nc.tensor.matmul(P2[:], xb, wlr[:, 512:514], start=True, stop=True)
            g_b = sbU.tile([N, H], f32, tag="g")
            nc.scalar.activation(g_b[:], P2[:], Act.Exp)
            el_src = eler[0:N, :] if not USE_FP8 else None
            elgs = []
            for h in range(H):
                elg = sbU.tile([N, D + 1], bf16, tag=f"elg{b}{h}",
                               name=f"elg{b}{h}", bufs=1)
                src = (el_src[:, h * D:(h + 1) * D] if el_src is not None
                       else state[("P1", b)][:, h * D:(h + 1) * D])
                nc.gpsimd.tensor_scalar(
                    elg[:, 0:D], src, g_b[:, h:h + 1], None, Alu.mult)
                nc.gpsimd.tensor_copy(elg[:, D:D + 1], g_b[:, h:h + 1])
                elgs.append(elg)
            state[("elg", b)] = elgs

        def emit_gather(b, h=None):
            # pT[j, i] per h; block m=0: j 0..24 runs, m=1: 24..48, m=2: 48..64
            staged = state[("staged", b)]
            if h is None:
                pT2 = sbU.tile([N, H * N], bf16, tag=f"pT2{b}",
                               name=f"pT2{b}", bufs=1)
                state[("pT", b, 0)] = pT2[:, 0:N]
                state[("pT", b, 1)] = pT2[:, N:2 * N]
                src1 = bass.AP(tensor=staged.tensor, offset=staged.offset,
                               ap=[[32 * 3072, 2], [1, 3072]])
                nc.sync.dma_start(pT2[0:48, :], src1)
                src2 = bass.AP(tensor=staged.tensor,
                               offset=staged.offset + 64 * 3072,
                               ap=[[32 * 3072, 1], [1, 2048]])
                nc.sync.dma_start(pT2[48:64, :], src2)
            else:
                pTh = sbU.tile([N, N], bf16, tag=f"pT{b}{h}",
                               name=f"pT{b}{h}", bufs=1)
                state[("pT", b, h)] = pTh[:]
                src1 = bass.AP(tensor=staged.tensor,
                               offset=staged.offset + 64 * h,
                               ap=[[32 * 3072, 2], [128, 24], [1, 64]])
                nc.sync.dma_start(pTh[0:48, :], src1)
                src2 = bass.AP(tensor=staged.tensor,
                               offset=staged.offset + 64 * 3072 + 64 * h,
                               ap=[[32 * 3072, 1], [128, 16], [1, 64]])
                nc.sync.dma_start(pTh[48:64, :], src2)

        def emit_tail(b):
            elgs = state[("elg", b)]
            t_parts = []
            for h in range(H):
                ag = psR.tile([N, D + 1], f32, tag="ru", name=f"ag{b}{h}")
                nc.tensor.matmul(ag[:], state[("pT", b, h)],
                                 elgs[h][:], start=True, stop=True)
                r_u = sbU.tile([N, 1], f32, tag="r")
                nc.vector.reciprocal(r_u[:], ag[:, D:D + 1])
                t_h = sbU.tile([N, D], f32, tag="th", name=f"th{h}")
                nc.vector.tensor_scalar(t_h[:], ag[:, 0:D], r_u[:], None,
                                        Alu.mult)
                t_parts.append(t_h)
            tsum = sbU.tile([N, D], f32, tag="tsum")
            nc.gpsimd.tensor_tensor(tsum[:], t_parts[0][:], t_parts[1][:],
                                    Alu.add)
            oT = psR.tile([D, N], f32, tag="ru", name=f"oT{b}")
            nc.tensor.transpose(oT[:], tsum[:], ident)
            nc.scalar.activation(y_all[:, N * b:N * (b + 1)], oT[:],
                                 Act.Identity, bias=bias_ap, scale=0.5)

        def emit_y(b):
            src = bass.AP(tensor=y_all.tensor, offset=y_all.offset + N * b,
                          ap=[[B_LOC * N, 128], [1, N]])
            dst = bass.AP(tensor=y_d.tensor, offset=y_d.offset + b * D * N,
                          ap=[[N, 128], [1, N]])
            nc.sync.dma_start(dst, src)

        for b in range(B_LOC):
            last = b == B_LOC - 1
            P1, eler, g_b = emit_head(b)
            emit_unit(b, 0, P1, eler, g_b)
            if last:
                pump(0)
            emit_mid(b, eler)
            if b >= 1:
                emit_tail(b - 1)
            if last:
                emit_gather(b, h=0)
            emit_unit(b, 1, P1, eler, g_b)
            pump(0)
            if last:
                emit_gather(b, h=1)
            else:
                emit_gather(b)
        # y for b 0..2 overlaps the last tail; y for b3 goes out last
        y_src = bass.AP(tensor=y_all.tensor, offset=y_all.offset,
                        ap=[[B_LOC * N, 128], [N, B_LOC - 1], [1, N]])
        y_dst = bass.AP(tensor=y_d.tensor, offset=y_d.offset,
                        ap=[[N, 128], [128 * N, B_LOC - 1], [1, N]])
        nc.sync.dma_start(y_dst, y_src)
        emit_tail(B_LOC - 1)
        emit_y(B_LOC - 1)

    nc.compile()
    _cache[key] = nc
    return nc


def _make_blobs(x, Wl, Wr, attn_a, bias):
    """Host-side prep: per-core blobA [128, NA] f32 + shared blobB."""
    import ml_dtypes
    x = np.asarray(x, np.float32)
    Wl = np.asarray(Wl, np.float32)
    Wr = np.asarray(Wr, np.float32)
    attn_a = np.asarray(attn_a, np.float32)
    bias = np.asarray(bias, np.float32)

    def pack_bf16(a):
        v = a.astype(ml_dtypes.bfloat16).view(np.uint16)
        return (v[:, 0::2].astype(np.uint32)
                | (v[:, 1::2].astype(np.uint32) << 16)).view(np.float32)

    wl_u = np.einsum("whd,hd->wh", Wl.reshape(W, H, D), attn_a) * NEG_SLOPE
    wlr = np.concatenate([Wl, Wr, wl_u], axis=1)                 # [128, 514]
    a_rep = np.concatenate(
        [np.repeat((0.8 * attn_a[h]).reshape(128, 1), 32, axis=1)
         for h in range(H)], axis=1)                             # [128, 64]
    bias_f = 0.5 * (bias.reshape(H, D)[0] + bias.reshape(H, D)[1])
    ident = np.zeros((128, N), np.float32)
    ident[0:N, :] = np.eye(N, dtype=np.float32)

    common = np.concatenate(
        [pack_bf16(wlr), pack_bf16(a_rep), bias_f.reshape(128, 1), ident],
        axis=1)

    # selector col order: col = 512c + loc; j = 8c + loc//64, i = loc%64
    loc = np.arange(512)
    CVM = [(c % 3, c // 3) if c < 6 else (c - 6, 2) for c in range(NCHUNK)]
    JB = [24 * m + 8 * v if m < 2 else 48 + 8 * v for (v, m) in CVM]
    if USE_FP8:
        sel = np.zeros((64, 2, 4096), np.float32)
        for c in range(NCHUNK):
            cols = 512 * c + loc
            sel[JB[c] + loc // 64, 0, cols] = 1.0
            sel[loc % 64, 1, cols] = 1.0
        s8 = sel.reshape(64, 8192).astype(ml_dtypes.float8_e4m3fn)
        blobB = s8.view(np.uint8).reshape(64, 2048, 4).copy().view(
            np.uint32).reshape(64, 2048).view(np.float32)
    else:
        s_sel = np.zeros((128, N * N), np.float32)
        for c in range(NCHUNK):
            cols = 512 * c + loc
            s_sel[JB[c] + loc // 64, cols] = 1.0
            s_sel[N + loc % 64, cols] = 1.0
        blobB = pack_bf16(s_sel)

    blobs = []
    for c in range(N_CORES):
        xs = x[c * B_LOC:(c + 1) * B_LOC]
        xsec = xs.transpose(1, 0, 2).reshape(128, B_LOC * N)
        blobA = np.concatenate([pack_bf16(xsec), common], axis=1)
        blobs.append((np.ascontiguousarray(blobA),
                      np.ascontiguousarray(blobB)))
    return blobs


def kernel(x, Wl, Wr, attn_a, bias):
    nc = _build()
    blobs = _make_blobs(x, Wl, Wr, attn_a, bias)
    in_maps = [{"blobA": blobs[c][0], "blobB": blobs[c][1]}
               for c in range(N_CORES)]
    res = run_bass_kernel_spmd(nc, in_maps, list(range(N_CORES)))
    out = np.concatenate([res.results[c]["y"] for c in range(N_CORES)],
                         axis=0)
    return out.astype(np.float32)
